# revision 9
# baseline (speedup 1.0000x reference)
"""Local (sliding-window) attention kernel for Trainium2, 8 NeuronCores.

Problem: B=2, S=2048, D=512, window=65 (halo 32 each side).
Sharding: 8 cores = batch(2) x sequence-chunks(4 x 512 queries).
Each core gets its 512 queries plus K/V rows [s0-32, s0+608) (zero-padded
at sequence edges, padded to 640 rows) and computes, per 128-query block:
    scores = Q @ K_win^T   (window of 192 keys, padded to 256 for f32r rate)
    softmax with additive band+edge mask
    out = attn @ V_win
Device outputs per core: out [512, 512] and the attention band [512, 192]
(the only nonzero region). The host scatters the band into the dense
[B, S, S] weights tensor (structural zeros never leave the device).
"""

import numpy as np

import concourse.bacc as bacc
import concourse.mybir as mybir
import concourse.tile as tile
from concourse.bass_utils import run_bass_kernel_spmd
from concourse.masks import make_identity

B, S, D = 2, 2048, 512
WINDOW, HALF = 65, 32
NCORES, NCHUNK = 8, 4
SQ = S // NCHUNK          # 512 queries per core
NB = SQ // 128            # 4 blocks of 128 queries
BANDW = 192               # key-window width per 128-block (128 + 2*32)
USE_F32R = False          # fp32r matmuls (4x PE rate, slightly rounded inputs)
KW = 256 if USE_F32R else BANDW  # f32r needs N>=256 for the fast path
KROWS = SQ + 2 * HALF + (128 - 2 * HALF)  # 640: halo'd K/V rows padded to x128
KCH = KROWS // 128        # 5 chunks of 128 K/V rows
SCALE = 1.0 / float(np.sqrt(D))
NEG = -1.0e30

_F32 = mybir.dt.float32
_F32R = mybir.dt.float32r


def _build_program():
    nc = bacc.Bacc(
        "TRN2", target_bir_lowering=False, debug=False, num_devices=NCORES
    )
    q = nc.dram_tensor("q", [SQ, D], _F32, kind="ExternalInput").ap()
    k = nc.dram_tensor("k", [KROWS, D], _F32, kind="ExternalInput").ap()
    v = nc.dram_tensor("v", [KROWS, D], _F32, kind="ExternalInput").ap()
    m = nc.dram_tensor("m", [NB, 128, KW], _F32, kind="ExternalInput").ap()
    out = nc.dram_tensor("out", [SQ, D], _F32, kind="ExternalOutput").ap()
    band = nc.dram_tensor("band", [SQ, BANDW], _F32, kind="ExternalOutput").ap()

    with tile.TileContext(nc) as tc:
        _emit(tc, q, k, v, m, out, band)
    nc.compile()
    return nc


def _emit(tc, q, k, v, m, out, band):
    nc = tc.nc
    Exp = mybir.ActivationFunctionType.Exp
    Ident = mybir.ActivationFunctionType.Identity

    with (
        tc.tile_pool(name="const", bufs=1) as cpool,
        tc.tile_pool(name="kv", bufs=1) as kvpool,
        tc.tile_pool(name="ld", bufs=3) as ldpool,
        tc.tile_pool(name="work", bufs=3) as wpool,
        tc.tile_pool(name="ps_tr", bufs=2, space="PSUM") as ps_tr,
        tc.tile_pool(name="ps_sc", bufs=2, space="PSUM") as ps_sc,
        tc.tile_pool(name="ps_at", bufs=2, space="PSUM") as ps_at,
        tc.tile_pool(name="ps_av", bufs=2, space="PSUM") as ps_av,
    ):
        ident = cpool.tile([128, 128], _F32)
        make_identity(nc, ident[:])

        mask_sb = cpool.tile([128, NB, KW], _F32)
        nc.sync.dma_start(mask_sb[:], m.rearrange("j r w -> r j w"))

        # V natural layout [k, d]: 5 chunks of 128 halo rows.
        v_sb = kvpool.tile([128, KCH, D], _F32)
        for c in range(KCH):
            nc.sync.dma_start(v_sb[:, c, :], v[c * 128 : (c + 1) * 128, :])

        # K transposed to [d, k]: kT[p, dc, x] = K_halo[x, dc*128 + p]
        kT = kvpool.tile([128, 4, KROWS], _F32)
        for c in range(KCH):
            k_in = ldpool.tile([128, D], _F32, tag="kin")
            nc.sync.dma_start(k_in[:], k[c * 128 : (c + 1) * 128, :])
            tp = ps_tr.tile([128, D], _F32, tag="tr")
            for dc in range(4):
                nc.tensor.transpose(
                    tp[:, dc * 128 : (dc + 1) * 128],
                    k_in[:, dc * 128 : (dc + 1) * 128],
                    ident[:],
                )
            nc.vector.tensor_copy(
                kT[:, :, c * 128 : (c + 1) * 128],
                tp[:].rearrange("p (a b) -> p a b", b=128),
            )

        for jb in range(NB):
            q_in = ldpool.tile([128, D], _F32, tag="qin")
            nc.sync.dma_start(q_in[:], q[jb * 128 : (jb + 1) * 128, :])
            qt_ps = ps_tr.tile([128, D], _F32, tag="tr")
            for dc in range(4):
                nc.tensor.transpose(
                    qt_ps[:, dc * 128 : (dc + 1) * 128],
                    q_in[:, dc * 128 : (dc + 1) * 128],
                    ident[:],
                )
            qT = wpool.tile([128, D], _F32, tag="qT")
            nc.vector.tensor_copy(qT[:], qt_ps[:])

            # scores[r, jl] = sum_d Q[jb*128+r, d] * K_halo[jb*128+jl, d]
            sc_ps = ps_sc.tile([128, KW], _F32)
            for dc in range(4):
                lhsT = qT[:, dc * 128 : (dc + 1) * 128]
                rhs = kT[:, dc, jb * 128 : jb * 128 + KW]
                if USE_F32R:
                    lhsT, rhs = lhsT.bitcast(_F32R), rhs.bitcast(_F32R)
                nc.tensor.matmul(
                    sc_ps[:], lhsT=lhsT, rhs=rhs,
                    start=(dc == 0), stop=(dc == 3),
                )

            msk = wpool.tile([128, KW], _F32, tag="msk")
            nc.vector.tensor_add(msk[:], sc_ps[:], mask_sb[:, jb, :])

            ex = wpool.tile([128, KW], _F32, tag="ex")
            den = wpool.tile([128, 1], _F32, tag="den")
            nc.scalar.activation(
                ex[:], msk[:], Exp, bias=0.0, scale=SCALE, accum_out=den[:]
            )
            rec = wpool.tile([128, 1], _F32, tag="rec")
            nc.vector.reciprocal(rec[:], den[:])

            attn = wpool.tile([128, BANDW], _F32, tag="attn")
            nc.scalar.activation(attn[:], ex[:, 0:BANDW], Ident, scale=rec[:])
            nc.sync.dma_start(band[jb * 128 : (jb + 1) * 128, :], attn[:])

            # attn^T (unnormalized exp^T) for the AV matmul: [k, q] layout.
            at_ps = ps_at.tile([128, 2, 128], _F32)
            nc.tensor.transpose(at_ps[:, 0, :], ex[:, 0:128], ident[:])
            nc.tensor.transpose(at_ps[0:64, 1, :], ex[:, 128:BANDW], ident[:])
            atT = wpool.tile([128, 2, 128], _F32, tag="atT")
            nc.vector.tensor_copy(atT[:, 0, :], at_ps[:, 0, :])
            nc.vector.tensor_copy(atT[0:64, 1, :], at_ps[0:64, 1, :])

            av_ps = ps_av.tile([128, D], _F32)
            lhs0, rhs0 = atT[:, 0, :], v_sb[:, jb, :]
            lhs1, rhs1 = atT[0:64, 1, :], v_sb[0:64, jb + 1, :]
            if USE_F32R:
                lhs0, rhs0 = lhs0.bitcast(_F32R), rhs0.bitcast(_F32R)
                lhs1, rhs1 = lhs1.bitcast(_F32R), rhs1.bitcast(_F32R)
            nc.tensor.matmul(av_ps[:], lhsT=lhs0, rhs=rhs0, start=True, stop=False)
            nc.tensor.matmul(av_ps[:], lhsT=lhs1, rhs=rhs1, start=False, stop=True)
            o_sb = wpool.tile([128, D], _F32, tag="o")
            nc.scalar.activation(o_sb[:], av_ps[:], Ident, scale=rec[:])
            nc.sync.dma_start(out[jb * 128 : (jb + 1) * 128, :], o_sb[:])


def _masks():
    """Per-chunk additive masks [NB, 128, KW]: 0 valid, NEG invalid."""
    r = np.arange(128)[:, None]
    jl = np.arange(KW)[None, :]
    band_ok = (jl - r >= 0) & (jl - r <= 2 * HALF)
    masks = []
    for c in range(NCHUNK):
        s0 = c * SQ
        per_block = []
        for jb in range(NB):
            g = s0 - HALF + jb * 128 + jl  # global key index, [1, KW]
            ok = band_ok & (g >= 0) & (g < S)
            per_block.append(np.where(ok, 0.0, NEG).astype(np.float32))
        masks.append(np.stack(per_block))
    return masks  # list of [NB, 128, KW] per chunk


_nc_cache = None


def _get_program():
    global _nc_cache
    if _nc_cache is None:
        _nc_cache = _build_program()
    return _nc_cache


def run_cores(query, key, value, trace=False, **kw):
    """Shard inputs, run the SPMD bass kernel, return per-core results."""
    nc = _get_program()
    masks = _masks()
    in_maps = []
    for core in range(NCORES):
        b, c = divmod(core, NCHUNK)
        s0 = c * SQ
        kpad = np.pad(np.asarray(key[b]), ((HALF, KROWS - SQ - HALF), (0, 0)))
        vpad = np.pad(np.asarray(value[b]), ((HALF, KROWS - SQ - HALF), (0, 0)))
        in_maps.append(
            {
                "q": np.ascontiguousarray(query[b, s0 : s0 + SQ]),
                "k": np.ascontiguousarray(kpad[s0 : s0 + KROWS]),
                "v": np.ascontiguousarray(vpad[s0 : s0 + KROWS]),
                "m": masks[c],
            }
        )
    return run_bass_kernel_spmd(
        nc, in_maps, list(range(NCORES)), trace=trace, **kw
    )


def kernel(query, key, value):
    query = np.asarray(query, dtype=np.float32)
    key = np.asarray(key, dtype=np.float32)
    value = np.asarray(value, dtype=np.float32)

    res = run_cores(query, key, value).results

    output = np.empty((B, S, D), dtype=np.float32)
    attn = np.zeros((B, S, S), dtype=np.float32)
    for core in range(NCORES):
        b, c = divmod(core, NCHUNK)
        s0 = c * SQ
        output[b, s0 : s0 + SQ] = res[core]["out"]
        bandv = res[core]["band"]
        for jb in range(NB):
            c0 = s0 + jb * 128 - HALF
            g_lo, g_hi = max(0, c0), min(S, c0 + BANDW)
            attn[b, s0 + jb * 128 : s0 + (jb + 1) * 128, g_lo:g_hi] = bandv[
                jb * 128 : (jb + 1) * 128, g_lo - c0 : g_hi - c0
            ]
    return output, attn
